# revision 1
# baseline (speedup 1.0000x reference)
"""EnhancedCrossAttention on 8 Trainium2 NeuronCores.

Sharding: core c = 4*b + g handles batch b and head-group g (4 of 16 heads).
Wq/Wk/Wv split column-wise per head group, Wo row-wise; partial outputs
summed on host (tensor-parallel allreduce done at gather time).

Per-core device kernel (all matmuls fp32r = e8m11, full PE rate):
  P1: QpT[256,1024]  = (Wq_g * scale)^T-projection, transposed layout
  P2: per kv-chunk(512): KpT[256,512] proj (transposed), Vp[512,260] proj
      (natural layout, 4 heads x (64 cols + ones col))
  P3: per chunk/head/kv-tile: S^T[128kv,1024q] = K Q^T; P^T = exp(obj*S^T)
      (obj rides the ACT per-partition scale); O^T[65,1024] += [V|1]^T P^T
      accumulated in PSUM per chunk, DVE-added into SBUF across chunks
      (row 64 = softmax denominator l)
  P4: per head: recip(l) -> PE ones-outer-product broadcast -> normalize;
      Y[1024,1024] = O^T.T @ Wo_g, DMA out.
"""

import numpy as np

DIM = 1024
H = 16
HD = 64
B = 2
NQ = 1024
NKV = 4096
HPG = 4           # heads per group (per core)
DH = HPG * HD     # 256 head-dim columns per core
NCORES = 8
KV_CHUNK = 512
N_CHUNKS = NKV // KV_CHUNK
KT = DIM // 128   # k-tiles over DIM

_prog_cache = {}


def _build(has_bq, has_bk, has_bv):
    key = (has_bq, has_bk, has_bv)
    if key in _prog_cache:
        return _prog_cache[key]

    import concourse.mybir as mybir
    import concourse.tile as tile
    from concourse import bacc

    f32 = mybir.dt.float32
    f32r = mybir.dt.float32r
    EXP = mybir.ActivationFunctionType.Exp
    MULT = mybir.AluOpType.mult

    nc = bacc.Bacc("TRN2")
    xqt = nc.dram_tensor("xqt", [DIM, NQ], f32r, kind="ExternalInput")
    xkt = nc.dram_tensor("xkt", [DIM, NKV], f32r, kind="ExternalInput")
    xvt = nc.dram_tensor("xvt", [DIM, NKV], f32r, kind="ExternalInput")
    wq = nc.dram_tensor("wq", [DIM, DH], f32r, kind="ExternalInput")
    wk = nc.dram_tensor("wk", [DIM, DH], f32r, kind="ExternalInput")
    wv = nc.dram_tensor("wv", [DIM, DH], f32r, kind="ExternalInput")
    wo = nc.dram_tensor("wo", [DH, DIM], f32r, kind="ExternalInput")
    obj = nc.dram_tensor("obj", [NKV], f32, kind="ExternalInput")
    bq = nc.dram_tensor("bq", [DH], f32, kind="ExternalInput") if has_bq else None
    bk = nc.dram_tensor("bk", [DH], f32, kind="ExternalInput") if has_bk else None
    bv = nc.dram_tensor("bv", [DH], f32r, kind="ExternalInput") if has_bv else None
    y = nc.dram_tensor("y", [NQ, DIM], f32, kind="ExternalOutput")

    with tile.TileContext(nc) as tc:
        with tc.tile_pool(name="const", bufs=1) as cpool:
            wq_sb = cpool.tile([128, KT, DH], f32r, tag="wq")
            wk_sb = cpool.tile([128, KT, DH], f32r, tag="wk")
            wv_sb = cpool.tile([128, KT, DH], f32r, tag="wv")
            wo_sb = cpool.tile([128, DH // 128, DIM], f32r, tag="wo")
            obj_sb = cpool.tile([128, NKV // 128], f32, tag="obj")
            ones_sb = cpool.tile([128, 128], f32r, tag="ones")
            qpt = cpool.tile([128, 2, NQ], f32r, tag="qpt")
            ot = cpool.tile([128, 2, NQ], f32r, tag="ot")
            oacc = [
                cpool.tile([65, NQ], f32, tag=f"oacc{h}", name=f"oacc{h}")
                for h in range(HPG)
            ]

            # DMA emission order matters for startup latency: wq first (P1
            # stationary), then per-k xq slices (P1 moving, pipelined), then
            # the rest in order of first use. wo is only needed at P4.
            nc.sync.dma_start(wq_sb[:], wq[:].rearrange("(k p) n -> p k n", p=128))
            nc.gpsimd.memset(ones_sb[:].bitcast(f32), 1.0)
            if has_bq:
                bq_sb = cpool.tile([128, 2], f32, tag="bq")
                nc.sync.dma_start(bq_sb[:], bq[:].rearrange("(m p) -> p m", p=128))
            if has_bk:
                bk_sb = cpool.tile([128, 2], f32, tag="bk")
                nc.sync.dma_start(bk_sb[:], bk[:].rearrange("(m p) -> p m", p=128))
            if has_bv:
                bv_sb = cpool.tile([1, DH], f32r, tag="bv")
                nc.sync.dma_start(bv_sb[:], bv[:].rearrange("(a n) -> a n", a=1))

            with (
                tc.tile_pool(name="pj", bufs=2, space="PSUM") as pjpool,
                tc.tile_pool(name="spsum", bufs=2, space="PSUM") as spool,
                tc.tile_pool(name="opsum", bufs=1, space="PSUM") as opool,
            ):
                # ---- P1: Q projection (transposed out) ----
                with tc.tile_pool(name="xq", bufs=1) as xqpool:
                    xq_sb = xqpool.tile([128, KT, NQ], f32r, tag="xq")
                    xq_re = xqt[:].rearrange("(k p) n -> p k n", p=128)
                    for k in range(KT):
                        nc.sync.dma_start(xq_sb[:, k, :], xq_re[:, k, :])
                    # wk right after xq: needed with chunk-0 xk for K
                    # proj; per-k slices so the first K-proj matmul only
                    # waits for 128KB of it
                    wk_re = wk[:].rearrange("(k p) n -> p k n", p=128)
                    for k in range(KT):
                        nc.sync.dma_start(wk_sb[:, k, :], wk_re[:, k, :])
                    for m in range(2):
                        for n in range(2):
                            ps = pjpool.tile([128, 512], f32, tag="pj")
                            for k in range(KT):
                                nc.tensor.matmul(
                                    ps[:],
                                    wq_sb[:, k, m * 128 : (m + 1) * 128],
                                    xq_sb[:, k, n * 512 : (n + 1) * 512],
                                    start=(k == 0),
                                    stop=(k == KT - 1),
                                )
                            dst = qpt[:, m, n * 512 : (n + 1) * 512]
                            if has_bq:
                                nc.vector.tensor_scalar_add(
                                    dst, ps[:], bq_sb[:, m : m + 1]
                                )
                            else:
                                nc.vector.tensor_copy(dst, ps[:])

                # ---- P2+P3: stream kv chunks; project K/V; attention ----
                with (
                    tc.tile_pool(name="xk", bufs=2) as xkpool,
                    tc.tile_pool(name="xv", bufs=2) as xvpool,
                    tc.tile_pool(name="kpt", bufs=2) as kptpool,
                    tc.tile_pool(name="vp", bufs=2) as vppool,
                    tc.tile_pool(name="pt", bufs=3) as ptpool,
                    tc.tile_pool(name="norm", bufs=2) as npool,
                ):
                    def normalize_head(h):
                        # recip of l (row 64) -> PE broadcast over 64
                        # partitions -> normalized O_h^T into ot
                        m = h // 2
                        rec = npool.tile([65, NQ], f32r, tag="rec", name="rec")
                        with nc.allow_low_precision("softmax recip rounding"):
                            nc.vector.reciprocal(rec[64:65, :], oacc[h][64:65, :])
                        otmp = None
                        if h % 2 == 1:
                            otmp = npool.tile([64, NQ], f32r, tag="otmp", name="otmp")
                        for n in range(2):
                            nsl = slice(n * 512, (n + 1) * 512)
                            psr = pjpool.tile([128, 512], f32, tag="pj", name="psr")
                            nc.tensor.matmul(
                                psr[0:64, :],
                                ones_sb[64:65, 0:64],
                                rec[64:65, nsl],
                                start=True,
                                stop=True,
                            )
                            dst = otmp[:, nsl] if h % 2 else ot[0:64, m, nsl]
                            nc.vector.tensor_tensor(
                                dst, oacc[h][0:64, nsl], psr[0:64, :], MULT
                            )
                        if h % 2 == 1:
                            for n in range(2):
                                nsl = slice(n * 512, (n + 1) * 512)
                                nc.sync.dma_start(ot[64:128, m, nsl], otmp[:, nsl])

                    xk_re = xkt[:].rearrange("(k p) n -> p k n", p=128)
                    xv_re = xvt[:].rearrange("(k p) n -> p k n", p=128)

                    def load_k(c):
                        cs = slice(c * KV_CHUNK, (c + 1) * KV_CHUNK)
                        xk_c = xkpool.tile(
                            [128, KT, KV_CHUNK], f32r, tag="xk", name="xk_c"
                        )
                        for k in range(KT):
                            nc.sync.dma_start(xk_c[:, k, :], xk_re[:, k, cs])
                        return xk_c

                    def load_v(c):
                        cs = slice(c * KV_CHUNK, (c + 1) * KV_CHUNK)
                        xv_c = xvpool.tile(
                            [128, KT, KV_CHUNK], f32r, tag="xv", name="xv_c"
                        )
                        for k in range(KT):
                            nc.sync.dma_start(xv_c[:, k, :], xv_re[:, k, cs])
                        return xv_c

                    ps_o = {}
                    prefetched = {0: (load_k(0), None)}
                    # consts not needed until after chunk-0 K-proj begins
                    nc.sync.dma_start(
                        wv_sb[:], wv[:].rearrange("(k p) n -> p k n", p=128)
                    )
                    nc.sync.dma_start(
                        obj_sb[:], obj[:].rearrange("(t p) -> p t", p=128)
                    )
                    for c in range(N_CHUNKS):
                        if c in prefetched:
                            xk_c, xv_c = prefetched.pop(c)
                        else:
                            xk_c, xv_c = load_k(c), load_v(c)
                        if c == 5:
                            # wo for P4: load in the tail of the streaming
                            # phase when DMA has spare bandwidth
                            nc.sync.dma_start(
                                wo_sb[:],
                                wo[:].rearrange("(t p) n -> p t n", p=128),
                            )
                        # K^T projection for this chunk
                        kpt_c = kptpool.tile([128, 2, KV_CHUNK], f32r, tag="kpt")
                        for m in range(2):
                            ps = pjpool.tile([128, 512], f32, tag="pj")
                            for k in range(KT):
                                nc.tensor.matmul(
                                    ps[:],
                                    wk_sb[:, k, m * 128 : (m + 1) * 128],
                                    xk_c[:, k, :],
                                    start=(k == 0),
                                    stop=(k == KT - 1),
                                )
                            if has_bk:
                                nc.vector.tensor_scalar_add(
                                    kpt_c[:, m, :], ps[:], bk_sb[:, m : m + 1]
                                )
                            else:
                                nc.vector.tensor_copy(kpt_c[:, m, :], ps[:])
                        if xv_c is None:
                            xv_c = load_v(c)
                        # V projection (natural layout + ones columns)
                        vp_c = vppool.tile([128, 4, HPG * 65], f32r, tag="vp")
                        nc.gpsimd.memset(vp_c[:].bitcast(f32), 1.0)
                        for t in range(4):
                            ps = pjpool.tile([128, 512], f32, tag="pj")
                            psv = ps[:, 0:DH]
                            for k in range(KT):
                                nc.tensor.matmul(
                                    psv,
                                    xv_c[:, k, t * 128 : (t + 1) * 128],
                                    wv_sb[:, k, :],
                                    start=(k == 0),
                                    stop=(k == KT - 1 and not has_bv),
                                )
                            if has_bv:
                                nc.tensor.matmul(
                                    psv,
                                    ones_sb[0:1, 0:128],
                                    bv_sb[0:1, :],
                                    start=False,
                                    stop=True,
                                )
                            nc.vector.tensor_copy(
                                vp_c[:, t, :].rearrange("p (h e) -> p h e", h=HPG)[
                                    :, :, 0:HD
                                ],
                                psv.rearrange("p (h e) -> p h e", h=HPG),
                            )
                        # attention on this chunk; in the last chunk run
                        # heads 3,2,1,0 so kt=1's heads normalize first (they
                        # feed the Y start-group below) and the critical last
                        # head is even (no cross-partition DMA hop)
                        h_order = (
                            [3, 2, 1, 0] if c == N_CHUNKS - 1 else range(HPG)
                        )
                        for h in h_order:
                            hb = (h % 2) * 64
                            m = h // 2
                            ps_o[h] = opool.tile(
                                [65, NQ], f32, tag="o", name=f"ps_o{h}"
                            )
                            for t in range(4):
                                ps_s = spool.tile([128, NQ], f32, tag="s")
                                for n in range(2):
                                    nc.tensor.matmul(
                                        ps_s[:, n * 512 : (n + 1) * 512],
                                        kpt_c[hb : hb + 64, m, t * 128 : (t + 1) * 128],
                                        qpt[hb : hb + 64, m, n * 512 : (n + 1) * 512],
                                        start=True,
                                        stop=True,
                                    )
                                pt_t = ptpool.tile([128, NQ], f32r, tag="pt")
                                ti = c * 4 + t
                                nc.scalar.activation(
                                    pt_t[:], ps_s[:], EXP,
                                    scale=obj_sb[:, ti : ti + 1],
                                )
                                for n in range(2):
                                    nc.tensor.matmul(
                                        ps_o[h][:, n * 512 : (n + 1) * 512],
                                        vp_c[:, t, h * 65 : (h + 1) * 65],
                                        pt_t[:, n * 512 : (n + 1) * 512],
                                        start=(t == 0),
                                        stop=(t == 3),
                                    )
                            if c == 0:
                                nc.vector.tensor_copy(oacc[h][:], ps_o[h][:])
                            else:
                                nc.vector.tensor_add(
                                    oacc[h][:], oacc[h][:], ps_o[h][:]
                                )
                            if c == N_CHUNKS - 1:
                                normalize_head(h)


                    # ---- P4b: Y = O^T.T @ Wo (reuses s psum) ----
                    with tc.tile_pool(name="yb", bufs=4) as ypool:
                        for mq in range(NQ // 128):
                            psy = spool.tile([128, NQ], f32, tag="s", name="psy")
                            for kt2 in (1, 0):
                                for n in range(2):
                                    nc.tensor.matmul(
                                        psy[:, n * 512 : (n + 1) * 512],
                                        ot[:, kt2, mq * 128 : (mq + 1) * 128],
                                        wo_sb[:, kt2, n * 512 : (n + 1) * 512],
                                        start=(kt2 == 1),
                                        stop=(kt2 == 0),
                                    )
                            yt = ypool.tile([128, NQ], f32, tag="yt")
                            nc.scalar.copy(yt[:], psy[:])
                            nc.sync.dma_start(
                                y[mq * 128 : (mq + 1) * 128, :], yt[:]
                            )

    nc.compile()
    _prog_cache[key] = nc
    return nc


def kernel(query, key, value, objectness_scores, Wq, bq, Wk, bk, Wv, bv, Wo, bo,
           _trace=False):
    from concourse.bass_utils import run_bass_kernel_spmd

    f = np.float32
    query = np.asarray(query, f)
    key_ = np.asarray(key, f)
    value = np.asarray(value, f)
    objs = np.asarray(objectness_scores, f)
    Wq = np.asarray(Wq, f); bq = np.asarray(bq, f)
    Wk = np.asarray(Wk, f); bk = np.asarray(bk, f)
    Wv = np.asarray(Wv, f); bv = np.asarray(bv, f)
    Wo = np.asarray(Wo, f); bo = np.asarray(bo, f)

    scale = np.float32(HD ** -0.5)
    has_bq = bool(np.any(bq)); has_bk = bool(np.any(bk)); has_bv = bool(np.any(bv))
    nc = _build(has_bq, has_bk, has_bv)

    in_maps = []
    for c in range(NCORES):
        b, g = divmod(c, NCORES // B)
        sl = slice(g * DH, (g + 1) * DH)
        m = {
            "xqt": np.ascontiguousarray(query[b].T),
            "xkt": np.ascontiguousarray(key_[b].T),
            "xvt": np.ascontiguousarray(value[b].T),
            "wq": np.ascontiguousarray(Wq[:, sl] * scale),
            "wk": np.ascontiguousarray(Wk[:, sl]),
            "wv": np.ascontiguousarray(Wv[:, sl]),
            "wo": np.ascontiguousarray(Wo[sl, :]),
            "obj": np.ascontiguousarray(objs[b]),
        }
        if has_bq:
            m["bq"] = np.ascontiguousarray(bq[sl] * scale)
        if has_bk:
            m["bk"] = np.ascontiguousarray(bk[sl])
        if has_bv:
            m["bv"] = np.ascontiguousarray(bv[sl])
        in_maps.append(m)

    res = run_bass_kernel_spmd(
        nc, in_maps, core_ids=list(range(NCORES)), trace=_trace
    )
    out = np.zeros((B, NQ, DIM), np.float64)
    for c in range(NCORES):
        out[c // (NCORES // B)] += res.results[c]["y"].astype(np.float64)
    out += bo.astype(np.float64)
    result = out.astype(np.float32)
    if _trace:
        return result, res
    return result



# revision 35
# speedup vs baseline: 1.2476x; 1.2476x over previous
"""EnhancedCrossAttention on 8 Trainium2 NeuronCores.

Sharding: core c = 4*b + g handles batch b and head-group g (4 of 16 heads).
Wq/Wk/Wv split column-wise per head group, Wo row-wise; partial outputs
summed on host (tensor-parallel allreduce done at gather time).

All operands bf16 (host-converted); psum accumulation f32.

Per-core device kernel:
  P1: QpT[256,1024] = (Wq_g*scale)^T-proj, k-outer into two psum tiles
  P2: per kv-chunk(512): KpT[256,512] proj (transposed), Vp[512,4x65] proj
      (natural layout, per-head 64 cols + ones col for the softmax denom)
  P3: per chunk/head/kv-tile t: S^T[128kv,1024q] = K Q^T (psum);
      P^T = exp(obj*S^T) -> bf16 (obj rides the ACT per-partition scale);
      O[q,65] += P^T-tile^T @ [V_h|1]  -- q-major: 8 q-tile matmuls of N=65
      accumulated in psum per (chunk,head, q-half), DVE-added into SBUF
      oacc (col 64 of each 65-block = softmax denominator l).
      Chunk c+1's K/V projections are interleaved into chunk c's attention
      stream so ACT (exp) never starves behind the 2-buffer score psum.
  P4: recip(l) -> per-partition tensor_scalar normalize -> bf16;
      PE transpose (vs identity) -> O^T; Y[1024,1024] = O^T.T @ Wo_g -> bf16.
"""

import numpy as np

DIM = 1024
H = 16
HD = 64
B = 2
NQ = 1024
NKV = 4096
HPG = 4           # heads per group (per core)
DH = HPG * HD     # 256 head-dim columns per core
NCORES = 8
KV_CHUNK = 512
N_CHUNKS = NKV // KV_CHUNK
KT = DIM // 128   # k-tiles over DIM

_prog_cache = {}


def _build(has_bq, has_bk, has_bv):
    key = (has_bq, has_bk, has_bv)
    if key in _prog_cache:
        return _prog_cache[key]

    import concourse.mybir as mybir
    import concourse.tile as tile
    from concourse import bacc

    f32 = mybir.dt.float32
    bf16 = mybir.dt.bfloat16
    EXP = mybir.ActivationFunctionType.Exp

    nc = bacc.Bacc("TRN2")
    xqt = nc.dram_tensor("xqt", [DIM, NQ], bf16, kind="ExternalInput")
    xkt = nc.dram_tensor("xkt", [DIM, NKV], bf16, kind="ExternalInput")
    xvt = nc.dram_tensor("xvt", [DIM, NKV], bf16, kind="ExternalInput")
    wq = nc.dram_tensor("wq", [DIM, DH], bf16, kind="ExternalInput")
    wk = nc.dram_tensor("wk", [DIM, DH], bf16, kind="ExternalInput")
    wv = nc.dram_tensor("wv", [DIM, DH], bf16, kind="ExternalInput")
    wo = nc.dram_tensor("wo", [DH, DIM], bf16, kind="ExternalInput")
    obj = nc.dram_tensor("obj", [NKV], f32, kind="ExternalInput")
    ident = nc.dram_tensor("ident", [128, 128], bf16, kind="ExternalInput")
    bq = nc.dram_tensor("bq", [DH], f32, kind="ExternalInput") if has_bq else None
    bk = nc.dram_tensor("bk", [DH], f32, kind="ExternalInput") if has_bk else None
    bv = nc.dram_tensor("bv", [DH], bf16, kind="ExternalInput") if has_bv else None
    y = nc.dram_tensor("y", [NQ, DIM], bf16, kind="ExternalOutput")

    with tile.TileContext(nc) as tc:
        with tc.tile_pool(name="const", bufs=1) as cpool:
            wq_sb = cpool.tile([128, KT, DH], bf16, tag="wq")
            wk_sb = cpool.tile([128, KT, DH], bf16, tag="wk")
            wv_sb = cpool.tile([128, KT, DH], bf16, tag="wv")
            wo_sb = cpool.tile([128, DH // 128, DIM], bf16, tag="wo")
            obj_sb = cpool.tile([128, NKV // 128], f32, tag="obj")
            id_sb = cpool.tile([128, 128], bf16, tag="ident")
            warm = cpool.tile([1, 512], bf16, tag="warm")
            qpt = cpool.tile([128, 2, NQ], bf16, tag="qpt")
            # oacc[:, h, half, 65*q + e]; col e=64 of each 65-block = l
            oacc = cpool.tile([128, HPG, 2, 4 * 65], f32, tag="oacc")
            rec = cpool.tile([128, HPG, 2, 4], f32, tag="rec")
            # onorm[:, qtile, 64*h + e] (normalized O, natural layout)
            onorm = cpool.tile([128, 8, DH], bf16, tag="onorm")
            ot = cpool.tile([128, 2, NQ], bf16, tag="ot")

            # startup DMA order: interleave xq/wq per-k so Q-proj (k-outer)
            # starts after slice 0 and is paced by DMA, then chunk-0 K/V.
            xq_re = xqt[:].rearrange("(k p) n -> p k n", p=128)
            with tc.tile_pool(name="xq", bufs=1) as xqpool:
                xq_sb = xqpool.tile([128, KT, NQ], bf16, tag="xq")
                nc.sync.dma_start(
                    wq_sb[:], wq[:].rearrange("(k p) n -> p k n", p=128)
                )
                for k in range(KT):
                    nc.sync.dma_start(xq_sb[:, k, :], xq_re[:, k, :])
                nc.sync.dma_start(
                    wk_sb[:], wk[:].rearrange("(k p) n -> p k n", p=128)
                )
                if has_bq:
                    bq_sb = cpool.tile([128, 2], f32, tag="bq")
                    nc.sync.dma_start(bq_sb[:], bq[:].rearrange("(m p) -> p m", p=128))
                if has_bk:
                    bk_sb = cpool.tile([128, 2], f32, tag="bk")
                    nc.sync.dma_start(bk_sb[:], bk[:].rearrange("(m p) -> p m", p=128))
                if has_bv:
                    bv_sb = cpool.tile([1, DH], bf16, tag="bv")
                    ones_sb = cpool.tile([1, 128], bf16, tag="ones")
                    nc.gpsimd.memset(ones_sb[:], 1.0)
                    nc.sync.dma_start(bv_sb[:], bv[:].rearrange("(a n) -> a n", a=1))

                xk_re = xkt[:].rearrange("(k p) n -> p k n", p=128)
                xv_re = xvt[:].rearrange("(k p) n -> p k n", p=128)

                with (
                    tc.tile_pool(name="spsum", bufs=2, space="PSUM") as spool,
                    tc.tile_pool(name="pj", bufs=2, space="PSUM") as pjpool,
                    tc.tile_pool(name="opsum", bufs=1, space="PSUM") as opool,
                    tc.tile_pool(name="xk", bufs=2) as xkpool,
                    tc.tile_pool(name="xv", bufs=2) as xvpool,
                    tc.tile_pool(name="kpt", bufs=2) as kptpool,
                    tc.tile_pool(name="vp", bufs=2) as vppool,
                    tc.tile_pool(name="pt", bufs=3) as ptpool,
                    tc.tile_pool(name="yb", bufs=3) as ypool,
                ):
                    def load_k(c, split=False):
                        cs = slice(c * KV_CHUNK, (c + 1) * KV_CHUNK)
                        xk_c = xkpool.tile(
                            [128, KT, KV_CHUNK], bf16, tag="xk", name="xk_c"
                        )
                        if split:
                            for k in range(KT):
                                nc.sync.dma_start(xk_c[:, k, :], xk_re[:, k, cs])
                        else:
                            nc.sync.dma_start(xk_c[:], xk_re[:, :, cs])
                        return xk_c

                    def load_v(c, split=False):
                        cs = slice(c * KV_CHUNK, (c + 1) * KV_CHUNK)
                        xv_c = xvpool.tile(
                            [128, KT, KV_CHUNK], bf16, tag="xv", name="xv_c"
                        )
                        if split:
                            for k in range(KT):
                                nc.sync.dma_start(xv_c[:, k, :], xv_re[:, k, cs])
                        else:
                            nc.sync.dma_start(xv_c[:], xv_re[:, :, cs])
                        return xv_c

                    xk0 = load_k(0, split=True)
                    nc.sync.dma_start(
                        obj_sb[:], obj[:].rearrange("(t p) -> p t", p=128)
                    )
                    nc.sync.dma_start(
                        wv_sb[:], wv[:].rearrange("(k p) n -> p k n", p=128)
                    )
                    xv0 = load_v(0, split=True)

                    # PE warmup: fills the initial DMA wait and completes the
                    # tensor-engine p-state ramp before real matmuls arrive.
                    # The dummy exp pulls the ACT table load (LoadActFuncSet,
                    # ~1.3us) out of the first real exp's critical path.
                    nc.gpsimd.memset(warm[:], 0.0)
                    wpo = opool.tile([128, 4 * 65], f32, tag="o0", name="wpo")
                    nc.tensor.matmul(
                        wpo[0:1, 0:260], warm[0:1, 0:1], warm[0:1, 0:260],
                        start=True, stop=True,
                    )
                    nc.scalar.activation(rec[0:1, 0, 0, 0:1], wpo[0:1, 0:1], EXP)
                    wps = pjpool.tile([128, 512], f32, tag="pj", name="wps")
                    for _ in range(6):
                        nc.tensor.matmul(
                            wps[0:1, :], warm[0:1, 0:1], warm[0:1, :],
                            start=True, stop=True,
                        )

                    # ---- P1: Q projection, k-outer, 2 psum tiles ----
                    qps = [
                        spool.tile([128, NQ], f32, tag="s", name=f"qps{m}")
                        for m in range(2)
                    ]
                    for k in range(KT):
                        for m in range(2):
                            for n in range(2):
                                nc.tensor.matmul(
                                    qps[m][:, n * 512 : (n + 1) * 512],
                                    wq_sb[:, k, m * 128 : (m + 1) * 128],
                                    xq_sb[:, k, n * 512 : (n + 1) * 512],
                                    start=(k == 0),
                                    stop=(k == KT - 1),
                                )
                    for m in range(2):
                        if has_bq:
                            nc.vector.tensor_scalar_add(
                                qpt[:, m, :], qps[m][:], bq_sb[:, m : m + 1]
                            )
                        else:
                            nc.vector.tensor_copy(qpt[:, m, :], qps[m][:])

                    # ---- P2 helpers ----
                    def proj_k_group(xk_c, kpt_c, m):
                        ps = pjpool.tile([128, 512], f32, tag="pj", name="pjk")
                        for k in range(KT):
                            nc.tensor.matmul(
                                ps[:],
                                wk_sb[:, k, m * 128 : (m + 1) * 128],
                                xk_c[:, k, :],
                                start=(k == 0),
                                stop=(k == KT - 1),
                            )
                        if has_bk:
                            nc.vector.tensor_scalar_add(
                                kpt_c[:, m, :], ps[:], bk_sb[:, m : m + 1]
                            )
                        else:
                            nc.vector.tensor_copy(kpt_c[:, m, :], ps[:])

                    def proj_v_group(xv_c, vp_c, t):
                        ps = pjpool.tile([128, 512], f32, tag="pj", name="pjv")
                        psv = ps[:, 0:DH]
                        for k in range(KT):
                            nc.tensor.matmul(
                                psv,
                                xv_c[:, k, t * 128 : (t + 1) * 128],
                                wv_sb[:, k, :],
                                start=(k == 0),
                                stop=(k == KT - 1 and not has_bv),
                            )
                        if has_bv:
                            nc.tensor.matmul(
                                psv,
                                ones_sb[0:1, 0:128],
                                bv_sb[0:1, :],
                                start=False,
                                stop=True,
                            )
                        nc.vector.tensor_copy(
                            vp_c[:, t, :].rearrange("p (h e) -> p h e", h=HPG)[
                                :, :, 0:HD
                            ],
                            psv.rearrange("p (h e) -> p h e", h=HPG),
                        )

                    def new_vp():
                        vp_c = vppool.tile([128, 4, HPG * 65], bf16, tag="vp")
                        # ones columns (col 64 of each 65-block) for the denom
                        nc.gpsimd.memset(
                            vp_c[:].rearrange("p t (h e) -> p t h e", h=HPG)[
                                :, :, :, 64:65
                            ],
                            1.0,
                        )
                        return vp_c

                    # chunk 0 projections (prologue)
                    kpt_c = kptpool.tile([128, 2, KV_CHUNK], bf16, tag="kpt")
                    vp_c = new_vp()
                    for m in range(2):
                        proj_k_group(xk0, kpt_c, m)
                    for t in range(4):
                        proj_v_group(xv0, vp_c, t)

                    # ---- P4 helper (also woven into the last chunk) ----
                    def transpose_pair(qt, p, act_copy=False):
                        tp = pjpool.tile([128, 512], f32, tag="pj", name="tp")
                        tpb = tp[:].bitcast(bf16)[:, 0:128]
                        nc.tensor.transpose(
                            tpb, onorm[:, qt, p * 128 : (p + 1) * 128], id_sb[:]
                        )
                        dst = ot[:, p, qt * 128 : (qt + 1) * 128]
                        if act_copy:
                            nc.scalar.copy(dst, tpb)
                        else:
                            nc.vector.tensor_copy(dst, tpb)

                    # ---- P3: attention over chunks, next-chunk proj woven in
                    cur_k, cur_v = kpt_c, vp_c
                    for c in range(N_CHUNKS):
                        kpt_c, vp_c = cur_k, cur_v
                        if c == 2:
                            nc.sync.dma_start(id_sb[:], ident[:])
                        if c + 1 < N_CHUNKS:
                            xk_n = load_k(c + 1, split=(c == 0))
                            xv_n = load_v(c + 1, split=(c == 0))
                            kpt_n = kptpool.tile(
                                [128, 2, KV_CHUNK], bf16, tag="kpt", name="kpt_n"
                            )
                            vp_n = new_vp()
                            proj_jobs = [
                                lambda m=m: proj_k_group(xk_n, kpt_n, m)
                                for m in range(2)
                            ] + [
                                lambda t=t: proj_v_group(xv_n, vp_n, t)
                                for t in range(4)
                            ]
                            cur_k, cur_v = kpt_n, vp_n
                        else:
                            proj_jobs = []
                        if c == 5:
                            nc.sync.dma_start(
                                wo_sb[:],
                                wo[:].rearrange("(t p) n -> p t n", p=128),
                            )

                        # flattened (h, t) steps, software-pipelined: emit
                        # S(i)+exp(i) two steps ahead of O(i)
                        steps = [(h, t) for h in range(HPG) for t in range(4)]
                        pt_tiles = {}
                        o_ps = {}

                        def emit_s(i):
                            h, t = steps[i]
                            hb = (h % 2) * 64
                            m = h // 2
                            ps_s = spool.tile([128, NQ], f32, tag="s", name="ps_s")
                            for n in range(2):
                                nc.tensor.matmul(
                                    ps_s[:, n * 512 : (n + 1) * 512],
                                    kpt_c[hb : hb + 64, m, t * 128 : (t + 1) * 128],
                                    qpt[hb : hb + 64, m, n * 512 : (n + 1) * 512],
                                    start=True,
                                    stop=True,
                                )
                            pt_t = ptpool.tile([128, NQ], bf16, tag="pt")
                            ti = c * 4 + t
                            nc.scalar.activation(
                                pt_t[:], ps_s[:], EXP,
                                scale=obj_sb[:, ti : ti + 1],
                            )
                            pt_tiles[i] = pt_t

                        def emit_o(i):
                            h, t = steps[i]
                            pt_t = pt_tiles.pop(i)
                            if t == 0:
                                o_ps[h] = [
                                    opool.tile(
                                        [128, 4 * 65], f32,
                                        tag=f"o{half}", name=f"o{half}",
                                    )
                                    for half in range(2)
                                ]
                            for half in range(2):
                                for q in range(4):
                                    # one accumulation group per psum bank:
                                    # start marks the whole bank pending-zero
                                    # (q>0 t==0 writes then overwrite, t>0
                                    # writes accumulate), stop on last write
                                    nc.tensor.matmul(
                                        o_ps[h][half][:, q * 65 : (q + 1) * 65],
                                        pt_t[:, (half * 4 + q) * 128 :
                                             (half * 4 + q + 1) * 128],
                                        vp_c[:, t, h * 65 : (h + 1) * 65],
                                        start=(t == 0 and q == 0),
                                        stop=(t == 3 and q == 3),
                                    )
                            if t == 3:
                                for half in range(2):
                                    dst = oacc[:, h, half, :]
                                    src = o_ps[h][half][:]
                                    # (GPSIMD cannot access PSUM, so both
                                    # halves go through DVE)
                                    if c == 0:
                                        nc.vector.tensor_copy(dst, src)
                                    else:
                                        nc.vector.tensor_add(dst, dst, src)
                                    if c == N_CHUNKS - 1:
                                        normalize_half(h, half)

                        def normalize_half(h, half):
                            # h==3 runs in the kernel tail where ACT is idle:
                            # split the multiplies across DVE and ACT there
                            use_act = h == 3
                            nc.vector.reciprocal(
                                rec[:, h, half, :],
                                oacc[:, h, half, :].rearrange(
                                    "p (q e) -> p q e", e=65
                                )[:, :, 64],
                            )
                            for q in range(4):
                                dst = onorm[:, half * 4 + q,
                                            h * HD : (h + 1) * HD]
                                src = oacc[:, h, half, q * 65 : q * 65 + 64]
                                sc = rec[:, h, half, q : q + 1]
                                if use_act and q % 2 == 1:
                                    nc.scalar.mul(dst, src, sc)
                                else:
                                    nc.vector.tensor_scalar_mul(dst, src, sc)

                        emit_s(0)
                        emit_s(1)
                        if proj_jobs:
                            fill_at = {2: 0, 4: 1, 6: 2, 9: 3, 11: 4, 13: 5}
                        else:
                            # last chunk: weave pair-0 transposes (heads 0/1
                            # normalized after step 7) into the ACT-bound tail
                            proj_jobs = [
                                lambda qt=qt: transpose_pair(qt, 0)
                                for qt in range(8)
                            ]
                            fill_at = {8 + j: j for j in range(8)}
                        for i in range(len(steps)):
                            emit_o(i)
                            if i + 2 < len(steps):
                                emit_s(i + 2)
                            if i in fill_at:
                                proj_jobs[fill_at[i]]()

                    # ---- P4: transpose pair 1; Y = O^T.T @ Wo ----
                    for qt in range(8):
                        transpose_pair(qt, 1, act_copy=(qt % 2 == 1))
                        psy = spool.tile([128, NQ], f32, tag="s", name="psy")
                        for kt2 in (0, 1):
                            for n in range(2):
                                nc.tensor.matmul(
                                    psy[:, n * 512 : (n + 1) * 512],
                                    ot[:, kt2, qt * 128 : (qt + 1) * 128],
                                    wo_sb[:, kt2, n * 512 : (n + 1) * 512],
                                    start=(kt2 == 0),
                                    stop=(kt2 == 1),
                                )
                        yt = ypool.tile([128, NQ], bf16, tag="yt")
                        qsl = slice(qt * 128, (qt + 1) * 128)
                        if qt >= 7:
                            # drain the final tile via parallel half-pipes
                            nc.scalar.copy(yt[:, 0:512], psy[:, 0:512])
                            nc.sync.dma_start(y[qsl, 0:512], yt[:, 0:512])
                            nc.vector.tensor_copy(yt[:, 512:NQ], psy[:, 512:NQ])
                            nc.sync.dma_start(y[qsl, 512:NQ], yt[:, 512:NQ])
                        elif qt % 2 == 0:
                            nc.scalar.copy(yt[:], psy[:])
                            nc.sync.dma_start(y[qsl, :], yt[:])
                        else:
                            nc.vector.tensor_copy(yt[:], psy[:])
                            nc.sync.dma_start(y[qsl, :], yt[:])

    nc.compile()
    _prog_cache[key] = nc
    return nc


def kernel(query, key, value, objectness_scores, Wq, bq, Wk, bk, Wv, bv, Wo, bo,
           _trace=False):
    import ml_dtypes
    from concourse.bass_utils import run_bass_kernel_spmd

    f = np.float32
    bf = ml_dtypes.bfloat16
    query = np.asarray(query, f)
    key_ = np.asarray(key, f)
    value = np.asarray(value, f)
    objs = np.asarray(objectness_scores, f)
    Wq = np.asarray(Wq, f); bq = np.asarray(bq, f)
    Wk = np.asarray(Wk, f); bk = np.asarray(bk, f)
    Wv = np.asarray(Wv, f); bv = np.asarray(bv, f)
    Wo = np.asarray(Wo, f); bo = np.asarray(bo, f)

    scale = np.float32(HD ** -0.5)
    has_bq = bool(np.any(bq)); has_bk = bool(np.any(bk)); has_bv = bool(np.any(bv))
    nc = _build(has_bq, has_bk, has_bv)

    ident = np.eye(128, dtype=bf)
    in_maps = []
    for c in range(NCORES):
        b, g = divmod(c, NCORES // B)
        sl = slice(g * DH, (g + 1) * DH)
        m = {
            "xqt": np.ascontiguousarray(query[b].T.astype(bf)),
            "xkt": np.ascontiguousarray(key_[b].T.astype(bf)),
            "xvt": np.ascontiguousarray(value[b].T.astype(bf)),
            "wq": np.ascontiguousarray((Wq[:, sl] * scale).astype(bf)),
            "wk": np.ascontiguousarray(Wk[:, sl].astype(bf)),
            "wv": np.ascontiguousarray(Wv[:, sl].astype(bf)),
            "wo": np.ascontiguousarray(Wo[sl, :].astype(bf)),
            "obj": np.ascontiguousarray(objs[b]),
            "ident": ident,
        }
        if has_bq:
            m["bq"] = np.ascontiguousarray(bq[sl] * scale)
        if has_bk:
            m["bk"] = np.ascontiguousarray(bk[sl])
        if has_bv:
            m["bv"] = np.ascontiguousarray(bv[sl].astype(bf))
        in_maps.append(m)

    res = run_bass_kernel_spmd(
        nc, in_maps, core_ids=list(range(NCORES)), trace=_trace
    )
    out = np.zeros((B, NQ, DIM), np.float64)
    for c in range(NCORES):
        out[c // (NCORES // B)] += res.results[c]["y"].astype(np.float64)
    out += bo.astype(np.float64)
    result = out.astype(np.float32)
    if _trace:
        return result, res
    return result


# revision 66
# speedup vs baseline: 1.2647x; 1.0137x over previous
"""EnhancedCrossAttention on 8 Trainium2 NeuronCores.

Sharding: core c = 4*b + g handles batch b and head-group g (4 of 16 heads).
Wq/Wk/Wv split column-wise per head group, Wo row-wise; partial outputs
summed on host (tensor-parallel allreduce done at gather time).

All operands bf16 (host-converted); psum accumulation f32.

Per-core device kernel:
  P1: QpT[256,1024] = (Wq_g*scale)^T-proj, k-outer into two psum tiles
  P2: per kv-chunk(512): KpT[256,512] proj (transposed), Vp[512,4x65] proj
      (natural layout, per-head 64 cols + ones col for the softmax denom)
  P3: per chunk/head/kv-tile t: S^T[128kv,1024q] = K Q^T (psum);
      P^T = exp(obj*S^T) -> bf16 (obj rides the ACT per-partition scale);
      O[q,65] += P^T-tile^T @ [V_h|1]  -- q-major: 8 q-tile matmuls of N=65
      accumulated in psum per (chunk,head, q-half), DVE-added into SBUF
      oacc (col 64 of each 65-block = softmax denominator l).
      Chunk c+1's K/V projections are interleaved into chunk c's attention
      stream so ACT (exp) never starves behind the 2-buffer score psum.
  P4: recip(l) -> per-partition tensor_scalar normalize -> bf16;
      PE transpose (vs identity) -> O^T; Y[1024,1024] = O^T.T @ Wo_g -> bf16.
"""

import numpy as np

DIM = 1024
H = 16
HD = 64
B = 2
NQ = 1024
NKV = 4096
HPG = 4           # heads per group (per core)
DH = HPG * HD     # 256 head-dim columns per core
NCORES = 8
KV_CHUNK = 512
N_CHUNKS = NKV // KV_CHUNK
KT = DIM // 128   # k-tiles over DIM

_prog_cache = {}


def _build(has_bq, has_bk, has_bv):
    key = (has_bq, has_bk, has_bv)
    if key in _prog_cache:
        return _prog_cache[key]

    import concourse.mybir as mybir
    import concourse.tile as tile
    from concourse import bacc

    f32 = mybir.dt.float32
    bf16 = mybir.dt.bfloat16
    fp8 = mybir.dt.float8e4
    DR = mybir.MatmulPerfMode.DoubleRow
    EXP = mybir.ActivationFunctionType.Exp
    # softmax is shift-invariant: exp(obj*s - ESH) keeps P within e4m3
    # range (max |logit| ~7.3 -> P' <= e^4.4 ~ 75 << 240)
    ESH = -3.0

    nc = bacc.Bacc("TRN2")
    xqt = nc.dram_tensor("xqt", [DIM, NQ], bf16, kind="ExternalInput")
    xkt = nc.dram_tensor("xkt", [DIM, NKV], bf16, kind="ExternalInput")
    xvt = nc.dram_tensor("xvt", [DIM, NKV], bf16, kind="ExternalInput")
    wq = nc.dram_tensor("wq", [DIM, DH], bf16, kind="ExternalInput")
    wk = nc.dram_tensor("wk", [DIM, DH], bf16, kind="ExternalInput")
    wv = nc.dram_tensor("wv", [DIM, DH], bf16, kind="ExternalInput")
    wo = nc.dram_tensor("wo", [DH, DIM], bf16, kind="ExternalInput")
    obj = nc.dram_tensor("obj", [NKV], f32, kind="ExternalInput")
    ident = nc.dram_tensor("ident", [128, 128], bf16, kind="ExternalInput")
    bq = nc.dram_tensor("bq", [DH], f32, kind="ExternalInput") if has_bq else None
    bk = nc.dram_tensor("bk", [DH], f32, kind="ExternalInput") if has_bk else None
    bv = nc.dram_tensor("bv", [DH], bf16, kind="ExternalInput") if has_bv else None
    y = nc.dram_tensor("y", [NQ, DIM], bf16, kind="ExternalOutput")

    with tile.TileContext(nc) as tc:
        with tc.tile_pool(name="const", bufs=1) as cpool:
            wq_sb = cpool.tile([128, KT, DH], bf16, tag="wq")
            wk_sb = cpool.tile([128, KT, DH], bf16, tag="wk")
            wv_sb = cpool.tile([128, KT, DH], bf16, tag="wv")
            wo_sb = cpool.tile([128, DH // 128, DIM], bf16, tag="wo")
            obj_sb = cpool.tile([128, NKV // 128], f32, tag="obj")
            id_sb = cpool.tile([128, 128], bf16, tag="ident")
            warm = cpool.tile([1, 512], bf16, tag="warm")
            esh_sb = cpool.tile([128, 1], f32, tag="esh")
            qpt = cpool.tile([128, 2, NQ], bf16, tag="qpt")
            # oacc[:, h, half, 65*q + e]; col e=64 of each 65-block = l
            oacc = cpool.tile([128, HPG, 2, 4 * 65], f32, tag="oacc")
            rec = cpool.tile([128, HPG, 2, 4], f32, tag="rec")
            # onorm[:, qtile, 64*h + e] (normalized O, natural layout)
            onorm = cpool.tile([128, 8, DH], bf16, tag="onorm")
            ot = cpool.tile([128, 2, NQ], bf16, tag="ot")

            # startup DMA order: interleave xq/wq per-k so Q-proj (k-outer)
            # starts after slice 0 and is paced by DMA, then chunk-0 K/V.
            xq_re = xqt[:].rearrange("(k p) n -> p k n", p=128)
            with tc.tile_pool(name="xq", bufs=1) as xqpool:
                xq_sb = xqpool.tile([128, KT, NQ], bf16, tag="xq")
                nc.sync.dma_start(
                    wq_sb[:], wq[:].rearrange("(k p) n -> p k n", p=128)
                )
                for k in range(KT):
                    nc.sync.dma_start(xq_sb[:, k, :], xq_re[:, k, :])
                nc.sync.dma_start(
                    wk_sb[:], wk[:].rearrange("(k p) n -> p k n", p=128)
                )
                if has_bq:
                    bq_sb = cpool.tile([128, 2], f32, tag="bq")
                    nc.sync.dma_start(bq_sb[:], bq[:].rearrange("(m p) -> p m", p=128))
                if has_bk:
                    bk_sb = cpool.tile([128, 2], f32, tag="bk")
                    nc.sync.dma_start(bk_sb[:], bk[:].rearrange("(m p) -> p m", p=128))
                if has_bv:
                    bv_sb = cpool.tile([1, DH], bf16, tag="bv")
                    ones_sb = cpool.tile([1, 128], bf16, tag="ones")
                    nc.gpsimd.memset(ones_sb[:], 1.0)
                    nc.sync.dma_start(bv_sb[:], bv[:].rearrange("(a n) -> a n", a=1))

                xk_re = xkt[:].rearrange("(k p) n -> p k n", p=128)
                xv_re = xvt[:].rearrange("(k p) n -> p k n", p=128)

                with (
                    tc.tile_pool(name="spsum", bufs=2, space="PSUM") as spool,
                    tc.tile_pool(name="pj", bufs=2, space="PSUM") as pjpool,
                    tc.tile_pool(name="opsum", bufs=1, space="PSUM") as opool,
                    tc.tile_pool(name="xk", bufs=2) as xkpool,
                    tc.tile_pool(name="xv", bufs=2) as xvpool,
                    tc.tile_pool(name="kpt", bufs=2) as kptpool,
                    tc.tile_pool(name="vp", bufs=2) as vppool,
                    tc.tile_pool(name="pt", bufs=3) as ptpool,
                    tc.tile_pool(name="yb", bufs=3) as ypool,
                ):
                    def load_k(c, split=False):
                        cs = slice(c * KV_CHUNK, (c + 1) * KV_CHUNK)
                        xk_c = xkpool.tile(
                            [128, KT, KV_CHUNK], bf16, tag="xk", name="xk_c"
                        )
                        if split:
                            for k in range(KT):
                                nc.sync.dma_start(xk_c[:, k, :], xk_re[:, k, cs])
                        else:
                            nc.sync.dma_start(xk_c[:], xk_re[:, :, cs])
                        return xk_c

                    def load_v(c, split=False):
                        cs = slice(c * KV_CHUNK, (c + 1) * KV_CHUNK)
                        xv_c = xvpool.tile(
                            [128, KT, KV_CHUNK], bf16, tag="xv", name="xv_c"
                        )
                        if split:
                            for k in range(KT):
                                nc.sync.dma_start(xv_c[:, k, :], xv_re[:, k, cs])
                        else:
                            nc.sync.dma_start(xv_c[:], xv_re[:, :, cs])
                        return xv_c

                    xk0 = load_k(0, split=True)
                    nc.sync.dma_start(
                        obj_sb[:], obj[:].rearrange("(t p) -> p t", p=128)
                    )
                    nc.sync.dma_start(
                        wv_sb[:], wv[:].rearrange("(k p) n -> p k n", p=128)
                    )
                    xv0 = load_v(0, split=True)

                    # PE warmup: fills the initial DMA wait and completes the
                    # tensor-engine p-state ramp before real matmuls arrive.
                    # The dummy exp pulls the ACT table load (LoadActFuncSet,
                    # ~1.3us) out of the first real exp's critical path.
                    nc.gpsimd.memset(warm[:], 0.0)
                    nc.gpsimd.memset(esh_sb[:], ESH)
                    wpo = opool.tile([128, 4 * 65], f32, tag="o0", name="wpo")
                    nc.tensor.matmul(
                        wpo[0:1, 0:260], warm[0:1, 0:1], warm[0:1, 0:260],
                        start=True, stop=True,
                    )
                    nc.scalar.activation(rec[0:1, 0, 0, 0:1], wpo[0:1, 0:1], EXP)
                    wps = pjpool.tile([128, 512], f32, tag="pj", name="wps")
                    for _ in range(6):
                        nc.tensor.matmul(
                            wps[0:1, :], warm[0:1, 0:1], warm[0:1, :],
                            start=True, stop=True,
                        )

                    # ---- P1: Q projection, k-outer per m-half ----
                    # only m=0 runs before attention; m=1 (heads 2/3, first
                    # needed at step 8) is deferred into the chunk-0 stream
                    qps = [
                        spool.tile([128, NQ], f32, tag="s", name=f"qps{m}")
                        for m in range(2)
                    ]
                    for k in range(KT):
                        for m in range(2):
                            for n in range(2):
                                nc.tensor.matmul(
                                    qps[m][:, n * 512 : (n + 1) * 512],
                                    wq_sb[:, k, m * 128 : (m + 1) * 128],
                                    xq_sb[:, k, n * 512 : (n + 1) * 512],
                                    start=(k == 0),
                                    stop=(k == KT - 1),
                                )
                    for m in range(2):
                        if has_bq:
                            nc.vector.tensor_scalar_add(
                                qpt[:, m, :], qps[m][:], bq_sb[:, m : m + 1]
                            )
                        else:
                            nc.vector.tensor_copy(qpt[:, m, :], qps[m][:])

                    # ---- P2 helpers ----
                    def proj_k_group(xk_c, kpt_c, m):
                        ps = pjpool.tile([128, 512], f32, tag="pj", name="pjk")
                        for k in range(KT):
                            nc.tensor.matmul(
                                ps[:],
                                wk_sb[:, k, m * 128 : (m + 1) * 128],
                                xk_c[:, k, :],
                                start=(k == 0),
                                stop=(k == KT - 1),
                            )
                        if has_bk:
                            nc.vector.tensor_scalar_add(
                                kpt_c[:, m, :], ps[:], bk_sb[:, m : m + 1]
                            )
                        else:
                            nc.vector.tensor_copy(kpt_c[:, m, :], ps[:])

                    def proj_v_group(xv_c, vp_c, t):
                        ps = pjpool.tile([128, 512], f32, tag="pj", name="pjv")
                        psv = ps[:, 0:DH]
                        for k in range(KT):
                            nc.tensor.matmul(
                                psv,
                                xv_c[:, k, t * 128 : (t + 1) * 128],
                                wv_sb[:, k, :],
                                start=(k == 0),
                                stop=(k == KT - 1 and not has_bv),
                            )
                        if has_bv:
                            nc.tensor.matmul(
                                psv,
                                ones_sb[0:1, 0:128],
                                bv_sb[0:1, :],
                                start=False,
                                stop=True,
                            )
                        nc.vector.tensor_copy(
                            vp_c[:, t, :].rearrange("p (h e) -> p h e", h=HPG)[
                                :, :, 0:HD
                            ],
                            psv.rearrange("p (h e) -> p h e", h=HPG),
                        )

                    def new_vp():
                        vp_c = vppool.tile([128, 4, HPG * 80], bf16, tag="vp")
                        # ones columns (col 64 of each block) for the denom
                        nc.gpsimd.memset(
                            vp_c[:].rearrange("p t (h e) -> p t h e", h=HPG)[
                                :, :, :, 64:65
                            ],
                            1.0,
                        )
                        return vp_c

                    # chunk 0 projections (prologue). Vp t2/t3 are deferred
                    # into chunk 0's attention stream (fill jobs) so the
                    # first exps start ~2us earlier.
                    kpt_c = kptpool.tile([128, 2, KV_CHUNK], bf16, tag="kpt")
                    vp_c = new_vp()
                    proj_k_group(xk0, kpt_c, 0)
                    proj_k_group(xk0, kpt_c, 1)
                    for t in range(2):
                        proj_v_group(xv0, vp_c, t)
                    # Vp t2/t3 deferred into chunk-0's attention stream
                    c0_jobs = [
                        lambda t=2: proj_v_group(xv0, vp_c, 2),
                        lambda t=3: proj_v_group(xv0, vp_c, 3),
                    ]

                    # ---- P4 helper (also woven into the last chunk) ----
                    def transpose_pair(qt, p, act_copy=False):
                        tp = pjpool.tile([128, 512], f32, tag="pj", name="tp")
                        tpb = tp[:].bitcast(bf16)[:, 0:128]
                        nc.tensor.transpose(
                            tpb, onorm[:, qt, p * 128 : (p + 1) * 128], id_sb[:]
                        )
                        dst = ot[:, p, qt * 128 : (qt + 1) * 128]
                        if act_copy:
                            nc.scalar.copy(dst, tpb)
                        else:
                            nc.vector.tensor_copy(dst, tpb)

                    # ---- P3: attention over chunks, next-chunk proj woven in
                    cur_k, cur_v = kpt_c, vp_c
                    for c in range(N_CHUNKS):
                        kpt_c, vp_c = cur_k, cur_v
                        if c == 2:
                            nc.sync.dma_start(id_sb[:], ident[:])
                        if c + 1 < N_CHUNKS:
                            xk_n = load_k(c + 1, split=(c == 0))
                            xv_n = load_v(c + 1, split=(c == 0))
                            kpt_n = kptpool.tile(
                                [128, 2, KV_CHUNK], bf16, tag="kpt", name="kpt_n"
                            )
                            vp_n = new_vp()
                            proj_jobs = [
                                lambda m=m: proj_k_group(xk_n, kpt_n, m)
                                for m in range(2)
                            ] + [
                                lambda t=t: proj_v_group(xv_n, vp_n, t)
                                for t in range(4)
                            ]
                            cur_k, cur_v = kpt_n, vp_n
                        else:
                            proj_jobs = []
                        if c == 5:
                            nc.sync.dma_start(
                                wo_sb[:],
                                wo[:].rearrange("(t p) n -> p t n", p=128),
                            )

                        # flattened (h, t) steps, software-pipelined: emit
                        # S(i)+exp(i) two steps ahead of O(i)
                        steps = [(h, t) for h in range(HPG) for t in range(4)]
                        pt_tiles = {}
                        o_ps = {}

                        def emit_s(i):
                            h, t = steps[i]
                            hb = (h % 2) * 64
                            m = h // 2
                            ps_s = spool.tile([128, NQ], f32, tag="s", name="ps_s")
                            for n in range(2):
                                nc.tensor.matmul(
                                    ps_s[:, n * 512 : (n + 1) * 512],
                                    kpt_c[hb : hb + 64, m, t * 128 : (t + 1) * 128],
                                    qpt[hb : hb + 64, m, n * 512 : (n + 1) * 512],
                                    start=True,
                                    stop=True,
                                )
                            # exp into one half of the current t-pair tile
                            if t % 2 == 0:
                                pt_tiles[i // 2] = ptpool.tile(
                                    [128, 2, NQ], bf16, tag="pt", name="pt_t"
                                )
                            pt_t = pt_tiles[i // 2]
                            ti = c * 4 + t
                            nc.scalar.activation(
                                pt_t[:, t % 2, :], ps_s[:], EXP,
                                bias=esh_sb[:, 0:1],
                                scale=obj_sb[:, ti : ti + 1],
                            )

                        def emit_o(i):
                            # processes a t-PAIR (the two exps of the current
                            # pt tile); called only on odd t
                            h, t = steps[i]
                            tp = t // 2
                            pt_t = pt_tiles.pop(i // 2)
                            if tp == 0:
                                o_ps[h] = [
                                    opool.tile(
                                        [128, 4 * 65], f32,
                                        tag=f"o{half}", name=f"o{half}",
                                    )
                                    for half in range(2)
                                ]
                            for half in range(2):
                                for q in range(4):
                                    for tt in range(2):
                                        # one accumulation group per psum
                                        # bank: start marks the whole bank
                                        # pending-zero (first writes then
                                        # overwrite, later ones accumulate)
                                        nc.tensor.matmul(
                                            o_ps[h][half][:,
                                                          q * 65 : (q + 1) * 65],
                                            pt_t[:, tt, (half * 4 + q) * 128 :
                                                 (half * 4 + q + 1) * 128],
                                            vp_c[:, tp * 2 + tt,
                                                 h * 80 : h * 80 + 65],
                                            start=(tp == 0 and q == 0
                                                   and tt == 0),
                                            stop=(tp == 1 and q == 3
                                                  and tt == 1),
                                        )
                            if t == 3:
                                for half in range(2):
                                    dst = oacc[:, h, half, :]
                                    src = o_ps[h][half][:]
                                    # (GPSIMD cannot access PSUM, so both
                                    # halves go through DVE)
                                    if c == 0:
                                        nc.vector.tensor_copy(dst, src)
                                    else:
                                        nc.vector.tensor_add(dst, dst, src)
                                    if c == N_CHUNKS - 1:
                                        normalize_half(h, half)

                        def normalize_half(h, half):
                            # h==3 runs in the kernel tail where ACT is idle:
                            # split the multiplies across DVE and ACT there
                            use_act = h == 3
                            nc.vector.reciprocal(
                                rec[:, h, half, :],
                                oacc[:, h, half, :].rearrange(
                                    "p (q e) -> p q e", e=65
                                )[:, :, 64],
                            )
                            for q in range(4):
                                dst = onorm[:, half * 4 + q,
                                            h * HD : (h + 1) * HD]
                                src = oacc[:, h, half, q * 65 : q * 65 + 64]
                                sc = rec[:, h, half, q : q + 1]
                                if use_act and q % 2 == 1:
                                    nc.scalar.mul(dst, src, sc)
                                else:
                                    nc.vector.tensor_scalar_mul(dst, src, sc)

                        emit_s(0)
                        emit_s(1)
                        if c == 0:
                            # deferred chunk-0 V groups (must land before
                            # their O-pairs at i=3), then chunk-1 work
                            fills = {
                                0: [c0_jobs[0]],
                                1: [c0_jobs[1]],
                                4: [proj_jobs[0]],
                                6: [proj_jobs[1]],
                                8: [proj_jobs[2]],
                                10: [proj_jobs[3]],
                                12: [proj_jobs[4]],
                                13: [proj_jobs[5]],
                            }
                        elif proj_jobs:
                            fills = {2: [proj_jobs[0]], 4: [proj_jobs[1]],
                                     6: [proj_jobs[2]], 9: [proj_jobs[3]],
                                     11: [proj_jobs[4]], 13: [proj_jobs[5]]}
                        else:
                            # last chunk: weave pair-0 transposes (heads 0/1
                            # normalized after step 7) into the ACT-bound tail
                            fills = {
                                8 + j: [lambda qt=j: transpose_pair(qt, 0)]
                                for j in range(8)
                            }
                        for i in range(len(steps)):
                            if steps[i][1] % 2 == 1:
                                emit_o(i)
                            if i + 2 < len(steps):
                                emit_s(i + 2)
                            for job in fills.get(i, ()):
                                job()

                    # ---- P4: transpose pair 1; Y = O^T.T @ Wo ----
                    # kt2=0 (pair 0, transposed during the last chunk) is
                    # emitted before the pair-1 transpose so the PE can run it
                    # while normalize(h3) is still draining
                    for qt in range(8):
                        psy = spool.tile([128, NQ], f32, tag="s", name="psy")
                        for n in range(2):
                            nc.tensor.matmul(
                                psy[:, n * 512 : (n + 1) * 512],
                                ot[:, 0, qt * 128 : (qt + 1) * 128],
                                wo_sb[:, 0, n * 512 : (n + 1) * 512],
                                start=True,
                                stop=False,
                            )
                        transpose_pair(qt, 1, act_copy=(qt % 2 == 1))
                        for n in range(2):
                            nc.tensor.matmul(
                                psy[:, n * 512 : (n + 1) * 512],
                                ot[:, 1, qt * 128 : (qt + 1) * 128],
                                wo_sb[:, 1, n * 512 : (n + 1) * 512],
                                start=False,
                                stop=True,
                            )
                        yt = ypool.tile([128, NQ], bf16, tag="yt")
                        qsl = slice(qt * 128, (qt + 1) * 128)
                        if qt >= 7:
                            # drain the final tile via parallel half-pipes
                            nc.scalar.copy(yt[:, 0:512], psy[:, 0:512])
                            nc.sync.dma_start(y[qsl, 0:512], yt[:, 0:512])
                            nc.vector.tensor_copy(yt[:, 512:NQ], psy[:, 512:NQ])
                            nc.sync.dma_start(y[qsl, 512:NQ], yt[:, 512:NQ])
                        elif qt % 2 == 0:
                            nc.scalar.copy(yt[:], psy[:])
                            nc.sync.dma_start(y[qsl, :], yt[:])
                        else:
                            nc.vector.tensor_copy(yt[:], psy[:])
                            nc.sync.dma_start(y[qsl, :], yt[:])

    nc.compile()
    _prog_cache[key] = nc
    return nc


def kernel(query, key, value, objectness_scores, Wq, bq, Wk, bk, Wv, bv, Wo, bo,
           _trace=False):
    import ml_dtypes
    from concourse.bass_utils import run_bass_kernel_spmd

    f = np.float32
    bf = ml_dtypes.bfloat16
    query = np.asarray(query, f)
    key_ = np.asarray(key, f)
    value = np.asarray(value, f)
    objs = np.asarray(objectness_scores, f)
    Wq = np.asarray(Wq, f); bq = np.asarray(bq, f)
    Wk = np.asarray(Wk, f); bk = np.asarray(bk, f)
    Wv = np.asarray(Wv, f); bv = np.asarray(bv, f)
    Wo = np.asarray(Wo, f); bo = np.asarray(bo, f)

    scale = np.float32(HD ** -0.5)
    has_bq = bool(np.any(bq)); has_bk = bool(np.any(bk)); has_bv = bool(np.any(bv))
    nc = _build(has_bq, has_bk, has_bv)

    ident = np.eye(128, dtype=bf)
    in_maps = []
    for c in range(NCORES):
        b, g = divmod(c, NCORES // B)
        sl = slice(g * DH, (g + 1) * DH)
        m = {
            "xqt": np.ascontiguousarray(query[b].T.astype(bf)),
            "xkt": np.ascontiguousarray(key_[b].T.astype(bf)),
            "xvt": np.ascontiguousarray(value[b].T.astype(bf)),
            "wq": np.ascontiguousarray((Wq[:, sl] * scale).astype(bf)),
            "wk": np.ascontiguousarray(Wk[:, sl].astype(bf)),
            "wv": np.ascontiguousarray(Wv[:, sl].astype(bf)),
            "wo": np.ascontiguousarray(Wo[sl, :].astype(bf)),
            "obj": np.ascontiguousarray(objs[b]),
            "ident": ident,
        }
        if has_bq:
            m["bq"] = np.ascontiguousarray(bq[sl] * scale)
        if has_bk:
            m["bk"] = np.ascontiguousarray(bk[sl])
        if has_bv:
            m["bv"] = np.ascontiguousarray(bv[sl].astype(bf))
        in_maps.append(m)

    res = run_bass_kernel_spmd(
        nc, in_maps, core_ids=list(range(NCORES)), trace=_trace
    )
    out = np.zeros((B, NQ, DIM), np.float64)
    for c in range(NCORES):
        out[c // (NCORES // B)] += res.results[c]["y"].astype(np.float64)
    out += bo.astype(np.float64)
    result = out.astype(np.float32)
    if _trace:
        return result, res
    return result
